# revision 3
# baseline (speedup 1.0000x reference)
"""Trainium2 Bass kernel for nn_AdaptiveTensorUnit.

Strategy (data-parallel over the unit axis N, per the sharding hint):
 - Shard the N=262144 units across 8 NeuronCores (32768 each).
 - Host-side prep (sharding/layout): compute integer cells, gather each
   unit's 128 local field samples (pure data staging), pre-transpose all
   per-unit tensors to feature-major [128, NS] layout so the device matmuls
   contract over partitions.
 - Device (per core): for both the old and offset position evaluations,
   run the 2-layer MLP on the TensorEngine (units on the moving/free dim,
   512 per tile), tanh + biases on ScalarE, squared-distance via a
   ones-matmul partition reduction, then accept/min/sqrt/select epilogue
   on VectorE/ScalarE. Outputs: stability [N] and final positions [N,3].
"""

import numpy as np
import ml_dtypes

N = 262144
G = 128
D = 128
NCORES = 8
NS = N // NCORES          # 32768 units per core
T = 512                   # units per matmul tile (moving dim)
NT = NS // T              # 64 tiles
UC = NS // 128            # 256 columns of 128 units

# Neighborhood offsets: first 128 of the 7x7x7 cube in i-major (i,j,k) order.
_grid = np.stack(
    np.meshgrid(np.arange(-3, 4), np.arange(-3, 4), np.arange(-3, 4), indexing="ij"),
    -1,
).reshape(-1, 3)[:D]
_OI = _grid[:, 0].astype(np.int32)
_OJ = _grid[:, 1].astype(np.int32)
_OK = _grid[:, 2].astype(np.int32)

_GRAPH = None


def _build_graph():
    import concourse.bass as bass
    import concourse.mybir as mybir
    import concourse.tile as tile
    from concourse import bacc

    f32 = mybir.dt.float32
    bf16 = mybir.dt.bfloat16
    f32r = mybir.dt.float32r
    AF = mybir.ActivationFunctionType
    ALU = mybir.AluOpType

    nc = bacc.Bacc(None, target_bir_lowering=False, debug=False)

    sigT = nc.dram_tensor("sigT", [128, NS], f32, kind="ExternalInput")
    sigbT = nc.dram_tensor("sigbT", [128, NS], bf16, kind="ExternalInput")
    locoldT = nc.dram_tensor("locoldT", [128, NS], bf16, kind="ExternalInput")
    locnewT = nc.dram_tensor("locnewT", [128, NS], bf16, kind="ExternalInput")
    w1t_d = nc.dram_tensor("W1top", [128, 128], bf16, kind="ExternalInput")
    w1b_d = nc.dram_tensor("W1bot", [128, 128], bf16, kind="ExternalInput")
    w2_d = nc.dram_tensor("W2c", [128, 128], bf16, kind="ExternalInput")
    b1_d = nc.dram_tensor("b1c", [128, 1], f32, kind="ExternalInput")
    b2_d = nc.dram_tensor("b2c", [128, 1], f32, kind="ExternalInput")
    posu_d = nc.dram_tensor("posu", [128, UC, 3], f32, kind="ExternalInput")
    offu_d = nc.dram_tensor("offu", [128, UC, 3], f32, kind="ExternalInput")
    stab_d = nc.dram_tensor("stab", [128, UC], f32, kind="ExternalOutput")
    fpos_d = nc.dram_tensor("fpos", [128, UC, 3], f32, kind="ExternalOutput")

    with tile.TileContext(nc) as tc:
        with (
            tc.tile_pool(name="singles", bufs=1) as singles,
            tc.tile_pool(name="persist", bufs=1) as persist,
            tc.tile_pool(name="stream", bufs=3) as stream,
            tc.tile_pool(name="work", bufs=3) as work,
            tc.tile_pool(name="psum_mm", bufs=3, space="PSUM") as psum_mm,
            tc.tile_pool(name="psum_d2", bufs=2, space="PSUM") as psum_d2,
        ):
            w1t = singles.tile([128, 128], bf16, tag="w1t")
            w1b = singles.tile([128, 128], bf16, tag="w1b")
            w2 = singles.tile([128, 128], bf16, tag="w2")
            b1 = singles.tile([128, 1], f32, tag="b1")
            b2 = singles.tile([128, 1], f32, tag="b2")
            ones = singles.tile([128, 1], f32, tag="ones")
            posu = singles.tile([128, UC, 3], f32, tag="posu")
            offu = singles.tile([128, UC, 3], f32, tag="offu")
            nc.sync.dma_start(out=w1t[:], in_=w1t_d[:])
            nc.sync.dma_start(out=w1b[:], in_=w1b_d[:])
            nc.sync.dma_start(out=w2[:], in_=w2_d[:])
            nc.sync.dma_start(out=b1[:], in_=b1_d[:])
            nc.sync.dma_start(out=b2[:], in_=b2_d[:])
            nc.sync.dma_start(out=posu[:], in_=posu_d[:])
            nc.sync.dma_start(out=offu[:], in_=offu_d[:])
            nc.vector.memset(ones[:], 1.0)

            d2o_all = persist.tile([128, UC], f32, tag="d2o")
            d2n_all = persist.tile([128, UC], f32, tag="d2n")
            acc_all = persist.tile([128, UC], f32, tag="acc")
            tmp_all = persist.tile([128, UC], f32, tag="tmp")
            stab_all = persist.tile([128, UC], f32, tag="stab")
            fp_all = persist.tile([128, UC, 3], f32, tag="fp")

            for t in range(NT):
                us = t * T
                sig_t = stream.tile([128, T], f32, tag="sig")
                nc.sync.dma_start(out=sig_t[:], in_=sigT[:, us : us + T])
                sigb_t = stream.tile([128, T], bf16, tag="sigb")
                nc.sync.dma_start(out=sigb_t[:], in_=sigbT[:, us : us + T])
                for loc_dram, d2_all in ((locoldT, d2o_all), (locnewT, d2n_all)):
                    loc_t = stream.tile([128, T], bf16, tag="loc")
                    nc.sync.dma_start(out=loc_t[:], in_=loc_dram[:, us : us + T])

                    pre = psum_mm.tile([128, T], f32, tag="mm")
                    nc.tensor.matmul(
                        pre[:], lhsT=w1t[:], rhs=sigb_t[:], start=True, stop=False
                    )
                    nc.tensor.matmul(
                        pre[:], lhsT=w1b[:], rhs=loc_t[:], start=False, stop=True
                    )
                    h_t = work.tile([128, T], bf16, tag="h")
                    nc.scalar.activation(h_t[:], pre[:], AF.Tanh, bias=b1[:])

                    resp = psum_mm.tile([128, T], f32, tag="mm")
                    nc.tensor.matmul(
                        resp[:], lhsT=w2[:], rhs=h_t[:], start=True, stop=True
                    )
                    diff_t = work.tile([128, T], f32, tag="diff")
                    nc.vector.scalar_tensor_tensor(
                        diff_t[:],
                        in0=resp[:],
                        scalar=b2[:],
                        in1=sig_t[:],
                        op0=ALU.add,
                        op1=ALU.subtract,
                    )
                    sq_t = work.tile([128, T], f32, tag="sq")
                    nc.vector.tensor_mul(sq_t[:], diff_t[:], diff_t[:])

                    d2p = psum_d2.tile([128, 4], f32, tag="d2")
                    for s in range(4):
                        nc.tensor.matmul(
                            d2p[:, s : s + 1],
                            lhsT=sq_t[:, s * 128 : (s + 1) * 128],
                            rhs=ones[:],
                            start=True,
                            stop=True,
                        )
                    nc.vector.tensor_copy(d2_all[:, t * 4 : (t + 1) * 4], d2p[:])

            # Epilogue: accept = (d2_new <= d2_old); stability = sqrt(min);
            # final_pos = pos + accept * offset.
            nc.vector.tensor_tensor(
                out=acc_all[:], in0=d2n_all[:], in1=d2o_all[:], op=ALU.is_le
            )
            nc.vector.tensor_tensor(
                out=tmp_all[:], in0=d2n_all[:], in1=d2o_all[:], op=ALU.min
            )
            nc.scalar.activation(stab_all[:], tmp_all[:], AF.Sqrt)
            nc.sync.dma_start(out=stab_d[:], in_=stab_all[:])
            for c in range(3):
                nc.vector.tensor_tensor(
                    out=fp_all[:, :, c],
                    in0=offu[:, :, c],
                    in1=acc_all[:],
                    op=ALU.mult,
                )
                nc.vector.tensor_tensor(
                    out=fp_all[:, :, c],
                    in0=fp_all[:, :, c],
                    in1=posu[:, :, c],
                    op=ALU.add,
                )
            nc.sync.dma_start(out=fpos_d[:], in_=fp_all[:])

    nc.finalize()
    return nc


def get_graph():
    global _GRAPH
    if _GRAPH is None:
        _GRAPH = _build_graph()
    return _GRAPH


def _locals_of(field, pos):
    pc = np.clip(pos.astype(np.int32), 0, G - 1)
    xx = np.clip(pc[:, 0:1] + _OI[None, :], 0, G - 1)
    yy = np.clip(pc[:, 1:2] + _OJ[None, :], 0, G - 1)
    zz = np.clip(pc[:, 2:3] + _OK[None, :], 0, G - 1)
    return field[xx, yy, zz]


def prepare_in_maps(universe_field, positions, signatures, offsets, W1, b1, W2, b2):
    field = np.asarray(universe_field, np.float32)
    pos = np.asarray(positions, np.float32)
    sig = np.asarray(signatures, np.float32)
    off = np.asarray(offsets, np.float32)
    W1 = np.asarray(W1, np.float32)
    b1 = np.asarray(b1, np.float32)
    W2 = np.asarray(W2, np.float32)
    b2 = np.asarray(b2, np.float32)
    test_pos = pos + off

    lo = _locals_of(field, pos)
    ln = _locals_of(field, test_pos)

    bf = ml_dtypes.bfloat16
    w1top = np.ascontiguousarray(W1[:D]).astype(bf)
    w1bot = np.ascontiguousarray(W1[D:]).astype(bf)
    w2c = np.ascontiguousarray(W2).astype(bf)
    b1c = b1.reshape(128, 1).astype(np.float32)
    b2c = b2.reshape(128, 1).astype(np.float32)

    in_maps = []
    for c in range(NCORES):
        sl = slice(c * NS, (c + 1) * NS)
        in_maps.append(
            {
                "sigT": np.ascontiguousarray(sig[sl].T),
                "sigbT": np.ascontiguousarray(sig[sl].T).astype(bf),
                "locoldT": np.ascontiguousarray(lo[sl].T).astype(bf),
                "locnewT": np.ascontiguousarray(ln[sl].T).astype(bf),
                "W1top": w1top,
                "W1bot": w1bot,
                "W2c": w2c,
                "b1c": b1c,
                "b2c": b2c,
                "posu": np.ascontiguousarray(
                    pos[sl].reshape(UC, 128, 3).transpose(1, 0, 2)
                ),
                "offu": np.ascontiguousarray(
                    off[sl].reshape(UC, 128, 3).transpose(1, 0, 2)
                ),
            }
        )
    return in_maps


def collect_outputs(results):
    stab = np.empty(N, np.float32)
    fp = np.empty((N, 3), np.float32)
    for c in range(NCORES):
        r = results[c]
        stab[c * NS : (c + 1) * NS] = np.asarray(r["stab"]).T.reshape(NS)
        fp[c * NS : (c + 1) * NS] = (
            np.asarray(r["fpos"]).transpose(1, 0, 2).reshape(NS, 3)
        )
    return stab, fp


def kernel(universe_field, positions, signatures, offsets, W1, b1, W2, b2):
    from concourse.bass_utils import run_bass_kernel_spmd

    in_maps = prepare_in_maps(
        universe_field, positions, signatures, offsets, W1, b1, W2, b2
    )
    nc = get_graph()
    res = run_bass_kernel_spmd(nc, in_maps, core_ids=list(range(NCORES)))
    return collect_outputs(res.results)


# revision 5
# speedup vs baseline: 1.0498x; 1.0498x over previous
"""Trainium2 Bass kernel for nn_AdaptiveTensorUnit.

Strategy (data-parallel over the unit axis N, per the sharding hint):
 - Shard the N=262144 units across 8 NeuronCores (32768 each).
 - Host-side prep (sharding/layout): compute integer cells, gather each
   unit's 128 local field samples (pure data staging), pre-transpose all
   per-unit tensors to feature-major [128, NS] layout so the device matmuls
   contract over partitions. Bias b2 is folded into the shipped signatures
   (sig* = sig - b2), standard constant folding.
 - Device (per core), per 512-unit tile and for both position evaluations:
     pre  = W1top.T @ sig*+b2(bf16) + W1bot.T @ local(bf16)      [PE]
     h    = tanh(pre + b1)                                        [ACT]
     dps  = W2.T @ h - sig*   (via -I stationary)                 [PE]
     sq   = dps * dps -> bf16                                     [DVE]
     d2   = sum over partitions (GPSIMD partition_all_reduce)
     row0 of the broadcast d2 -> DRAM scratch                     [DMA]
   Epilogue: reload d2 as [128, 256] columns, accept = d2n <= d2o,
   stability = sqrt(min), final_pos = pos + accept*offset, DMA out.
"""

import numpy as np
import ml_dtypes

N = 262144
G = 128
D = 128
NCORES = 8
NS = N // NCORES          # 32768 units per core
T = 512                   # units per matmul tile (moving dim)
NT = NS // T              # 64 tiles
UC = NS // 128            # 256 columns of 128 units

# Neighborhood offsets: first 128 of the 7x7x7 cube in i-major (i,j,k) order.
_grid = np.stack(
    np.meshgrid(np.arange(-3, 4), np.arange(-3, 4), np.arange(-3, 4), indexing="ij"),
    -1,
).reshape(-1, 3)[:D]
_OI = _grid[:, 0].astype(np.int32)
_OJ = _grid[:, 1].astype(np.int32)
_OK = _grid[:, 2].astype(np.int32)

_GRAPH = None


def _build_graph():
    import concourse.bass as bass
    import concourse.mybir as mybir
    import concourse.tile as tile
    import concourse.bass_isa as bass_isa
    from concourse import bacc

    f32 = mybir.dt.float32
    bf16 = mybir.dt.bfloat16
    AF = mybir.ActivationFunctionType
    ALU = mybir.AluOpType

    nc = bacc.Bacc(None, target_bir_lowering=False, debug=False)

    sigbT = nc.dram_tensor("sigbT", [128, NS], bf16, kind="ExternalInput")
    locoldT = nc.dram_tensor("locoldT", [128, NS], bf16, kind="ExternalInput")
    locnewT = nc.dram_tensor("locnewT", [128, NS], bf16, kind="ExternalInput")
    w1t_d = nc.dram_tensor("W1top", [128, 128], bf16, kind="ExternalInput")
    w1b_d = nc.dram_tensor("W1bot", [128, 128], bf16, kind="ExternalInput")
    w2_d = nc.dram_tensor("W2c", [128, 128], bf16, kind="ExternalInput")
    negI_d = nc.dram_tensor("negI", [128, 128], bf16, kind="ExternalInput")
    b1_d = nc.dram_tensor("b1c", [128, 1], f32, kind="ExternalInput")
    posu_d = nc.dram_tensor("posu", [128, UC, 3], f32, kind="ExternalInput")
    offu_d = nc.dram_tensor("offu", [128, UC, 3], f32, kind="ExternalInput")
    stab_d = nc.dram_tensor("stab", [128, UC], f32, kind="ExternalOutput")
    fpos_d = nc.dram_tensor("fpos", [128, UC, 3], f32, kind="ExternalOutput")
    d2o_d = nc.dram_tensor("d2o_scratch", [128, UC], f32)
    d2n_d = nc.dram_tensor("d2n_scratch", [128, UC], f32)

    with tile.TileContext(nc) as tc:
        with (
            tc.tile_pool(name="singles", bufs=1) as singles,
            tc.tile_pool(name="persist", bufs=1) as persist,
            tc.tile_pool(name="stream", bufs=3) as stream,
            tc.tile_pool(name="work", bufs=3) as work,
            tc.tile_pool(name="psum_mm", bufs=6, space="PSUM") as psum_mm,
        ):
            w1t = singles.tile([128, 128], bf16, tag="w1t")
            w1b = singles.tile([128, 128], bf16, tag="w1b")
            w2 = singles.tile([128, 128], bf16, tag="w2")
            negI = singles.tile([128, 128], bf16, tag="negI")
            b1 = singles.tile([128, 1], f32, tag="b1")
            posu = singles.tile([128, UC, 3], f32, tag="posu")
            offu = singles.tile([128, UC, 3], f32, tag="offu")
            nc.sync.dma_start(out=w1t[:], in_=w1t_d[:])
            nc.sync.dma_start(out=w1b[:], in_=w1b_d[:])
            nc.sync.dma_start(out=w2[:], in_=w2_d[:])
            nc.sync.dma_start(out=negI[:], in_=negI_d[:])
            nc.sync.dma_start(out=b1[:], in_=b1_d[:])
            nc.sync.dma_start(out=posu[:], in_=posu_d[:])
            nc.sync.dma_start(out=offu[:], in_=offu_d[:])

            for t in range(NT):
                us = t * T
                sigb_t = stream.tile([128, T], bf16, tag="sigb")
                nc.sync.dma_start(out=sigb_t[:], in_=sigbT[:, us : us + T])
                loco_t = stream.tile([128, T], bf16, tag="loco")
                nc.sync.dma_start(out=loco_t[:], in_=locoldT[:, us : us + T])
                locn_t = stream.tile([128, T], bf16, tag="locn")
                nc.sync.dma_start(out=locn_t[:], in_=locnewT[:, us : us + T])

                # Layer 1, stationaries shared across both evals.
                preO = psum_mm.tile([128, T], f32, tag="mm")
                preN = psum_mm.tile([128, T], f32, tag="mm")
                nc.tensor.matmul(preO[:], lhsT=w1t[:], rhs=sigb_t[:], start=True, stop=False)
                nc.tensor.matmul(preN[:], lhsT=w1t[:], rhs=sigb_t[:], start=True, stop=False)
                nc.tensor.matmul(preO[:], lhsT=w1b[:], rhs=loco_t[:], start=False, stop=True)
                nc.tensor.matmul(preN[:], lhsT=w1b[:], rhs=locn_t[:], start=False, stop=True)

                hO = work.tile([128, T], bf16, tag="h")
                hN = work.tile([128, T], bf16, tag="h")
                nc.scalar.activation(hO[:], preO[:], AF.Tanh, bias=b1[:])
                nc.scalar.activation(hN[:], preN[:], AF.Tanh, bias=b1[:])

                # Layer 2 minus signatures: dps = W2.T @ h - sig*
                dpsO = psum_mm.tile([128, T], f32, tag="mm")
                dpsN = psum_mm.tile([128, T], f32, tag="mm")
                nc.tensor.matmul(dpsO[:], lhsT=w2[:], rhs=hO[:], start=True, stop=False)
                nc.tensor.matmul(dpsN[:], lhsT=w2[:], rhs=hN[:], start=True, stop=False)
                nc.tensor.matmul(dpsO[:], lhsT=negI[:], rhs=sigb_t[:], start=False, stop=True)
                nc.tensor.matmul(dpsN[:], lhsT=negI[:], rhs=sigb_t[:], start=False, stop=True)

                # Square: one eval on ScalarE (single PSUM read), the other on
                # DVE via copy+mul — balances the two elementwise engines.
                sqO = work.tile([128, T], bf16, tag="sq")
                sqN = work.tile([128, T], bf16, tag="sq")
                nc.scalar.activation(sqO[:], dpsO[:], AF.Square)
                dcp = work.tile([128, T], f32, tag="dcp")
                nc.vector.tensor_copy(dcp[:], dpsN[:])
                nc.vector.tensor_mul(sqN[:], dcp[:], dcp[:])

                d2bO = work.tile([128, T], f32, tag="d2b")
                d2bN = work.tile([128, T], f32, tag="d2b")
                nc.gpsimd.partition_all_reduce(
                    d2bO[:], sqO[:], channels=128, reduce_op=bass_isa.ReduceOp.add
                )
                nc.gpsimd.partition_all_reduce(
                    d2bN[:], sqN[:], channels=128, reduce_op=bass_isa.ReduceOp.add
                )
                nc.sync.dma_start(out=d2o_d[2 * t : 2 * t + 2, :], in_=d2bO[0:1, :])
                nc.sync.dma_start(out=d2n_d[2 * t : 2 * t + 2, :], in_=d2bN[0:1, :])

            # Epilogue: accept, stability, final positions.
            d2o_sb = persist.tile([128, UC], f32, tag="d2o")
            d2n_sb = persist.tile([128, UC], f32, tag="d2n")
            acc_sb = persist.tile([128, UC], f32, tag="acc")
            min_sb = persist.tile([128, UC], f32, tag="min")
            stab_sb = persist.tile([128, UC], f32, tag="stab")
            fp_sb = persist.tile([128, UC, 3], f32, tag="fp")
            nc.sync.dma_start(out=d2o_sb[:], in_=d2o_d[:])
            nc.sync.dma_start(out=d2n_sb[:], in_=d2n_d[:])
            nc.vector.tensor_tensor(
                out=acc_sb[:], in0=d2n_sb[:], in1=d2o_sb[:], op=ALU.is_le
            )
            nc.vector.tensor_tensor(
                out=min_sb[:], in0=d2n_sb[:], in1=d2o_sb[:], op=ALU.min
            )
            nc.scalar.activation(stab_sb[:], min_sb[:], AF.Sqrt)
            nc.sync.dma_start(out=stab_d[:], in_=stab_sb[:])
            for c in range(3):
                nc.vector.tensor_tensor(
                    out=fp_sb[:, :, c], in0=offu[:, :, c], in1=acc_sb[:], op=ALU.mult
                )
                nc.vector.tensor_tensor(
                    out=fp_sb[:, :, c], in0=fp_sb[:, :, c], in1=posu[:, :, c], op=ALU.add
                )
            nc.sync.dma_start(out=fpos_d[:], in_=fp_sb[:])

    nc.finalize()
    return nc


def get_graph():
    global _GRAPH
    if _GRAPH is None:
        _GRAPH = _build_graph()
    return _GRAPH


def _locals_of(field, pos):
    pc = np.clip(pos.astype(np.int32), 0, G - 1)
    xx = np.clip(pc[:, 0:1] + _OI[None, :], 0, G - 1)
    yy = np.clip(pc[:, 1:2] + _OJ[None, :], 0, G - 1)
    zz = np.clip(pc[:, 2:3] + _OK[None, :], 0, G - 1)
    return field[xx, yy, zz]


def prepare_in_maps(universe_field, positions, signatures, offsets, W1, b1, W2, b2):
    field = np.asarray(universe_field, np.float32)
    pos = np.asarray(positions, np.float32)
    sig = np.asarray(signatures, np.float32)
    off = np.asarray(offsets, np.float32)
    W1 = np.asarray(W1, np.float32)
    b1 = np.asarray(b1, np.float32)
    W2 = np.asarray(W2, np.float32)
    b2 = np.asarray(b2, np.float32)
    test_pos = pos + off

    lo = _locals_of(field, pos)
    ln = _locals_of(field, test_pos)
    sigstar = sig - b2[None, :]

    bf = ml_dtypes.bfloat16
    w1top = np.ascontiguousarray(W1[:D]).astype(bf)
    w1bot = np.ascontiguousarray(W1[D:]).astype(bf)
    w2c = np.ascontiguousarray(W2).astype(bf)
    negI = (-np.eye(128, dtype=np.float32)).astype(bf)
    b1c = b1.reshape(128, 1).astype(np.float32)

    in_maps = []
    for c in range(NCORES):
        sl = slice(c * NS, (c + 1) * NS)
        in_maps.append(
            {
                "sigbT": np.ascontiguousarray(sigstar[sl].T).astype(bf),
                "locoldT": np.ascontiguousarray(lo[sl].T).astype(bf),
                "locnewT": np.ascontiguousarray(ln[sl].T).astype(bf),
                "W1top": w1top,
                "W1bot": w1bot,
                "W2c": w2c,
                "negI": negI,
                "b1c": b1c,
                "posu": np.ascontiguousarray(pos[sl].reshape(128, UC, 3)),
                "offu": np.ascontiguousarray(off[sl].reshape(128, UC, 3)),
            }
        )
    return in_maps


def collect_outputs(results):
    stab = np.empty(N, np.float32)
    fp = np.empty((N, 3), np.float32)
    for c in range(NCORES):
        r = results[c]
        stab[c * NS : (c + 1) * NS] = np.asarray(r["stab"]).reshape(NS)
        fp[c * NS : (c + 1) * NS] = np.asarray(r["fpos"]).reshape(NS, 3)
    return stab, fp


def kernel(universe_field, positions, signatures, offsets, W1, b1, W2, b2):
    from concourse.bass_utils import run_bass_kernel_spmd

    in_maps = prepare_in_maps(
        universe_field, positions, signatures, offsets, W1, b1, W2, b2
    )
    nc = get_graph()
    res = run_bass_kernel_spmd(nc, in_maps, core_ids=list(range(NCORES)))
    return collect_outputs(res.results)


# revision 12
# speedup vs baseline: 1.8607x; 1.7723x over previous
"""Trainium2 Bass kernel for nn_AdaptiveTensorUnit.

Strategy (data-parallel over the unit axis N, per the sharding hint):
 - Shard the N=262144 units across 8 NeuronCores (32768 each).
 - Host-side prep (sharding/layout): compute integer cells, gather each
   unit's 128 local field samples (pure data staging), pre-transpose all
   per-unit tensors to feature-major [128, NS] layout so the device matmuls
   contract over partitions. Bias b2 is folded into the shipped signatures
   (sig* = sig - b2), standard constant folding.
 - Device (per core), per 512-unit tile and for both position evaluations:
     pre  = W1top.T @ sig*+b2(bf16) + W1bot.T @ local(bf16)      [PE]
     h    = tanh(pre + b1)                                        [ACT]
     dps  = W2.T @ h - sig*   (via -I stationary)                 [PE]
     sq   = dps * dps -> bf16                                     [DVE]
     d2   = sum over partitions (GPSIMD partition_all_reduce)
     row0 of the broadcast d2 -> DRAM scratch                     [DMA]
   Epilogue: reload d2 as [128, 256] columns, accept = d2n <= d2o,
   stability = sqrt(min), final_pos = pos + accept*offset, DMA out.
"""

import numpy as np
import ml_dtypes

N = 262144
G = 128
D = 128
NCORES = 8
NS = N // NCORES          # 32768 units per core
T = 512                   # units per matmul tile (moving dim)
NT = NS // T              # 64 tiles
UC = NS // 128            # 256 columns of 128 units

# Neighborhood offsets: first 128 of the 7x7x7 cube in i-major (i,j,k) order.
_grid = np.stack(
    np.meshgrid(np.arange(-3, 4), np.arange(-3, 4), np.arange(-3, 4), indexing="ij"),
    -1,
).reshape(-1, 3)[:D]
_OI = _grid[:, 0].astype(np.int32)
_OJ = _grid[:, 1].astype(np.int32)
_OK = _grid[:, 2].astype(np.int32)

_GRAPH = None


def _build_graph():
    import concourse.bass as bass
    import concourse.mybir as mybir
    import concourse.tile as tile
    import concourse.bass_isa as bass_isa
    from concourse import bacc

    f32 = mybir.dt.float32
    bf16 = mybir.dt.bfloat16
    AF = mybir.ActivationFunctionType
    ALU = mybir.AluOpType

    nc = bacc.Bacc(None, target_bir_lowering=False, debug=False)

    sigbT = nc.dram_tensor("sigbT", [128, NS], bf16, kind="ExternalInput")
    locoldT = nc.dram_tensor("locoldT", [128, NS], bf16, kind="ExternalInput")
    locnewT = nc.dram_tensor("locnewT", [128, NS], bf16, kind="ExternalInput")
    w1t_d = nc.dram_tensor("W1top", [128, 128], bf16, kind="ExternalInput")
    w1b_d = nc.dram_tensor("W1bot", [128, 128], bf16, kind="ExternalInput")
    w2_d = nc.dram_tensor("W2c", [128, 128], bf16, kind="ExternalInput")
    negI_d = nc.dram_tensor("negI", [128, 128], bf16, kind="ExternalInput")
    onesm_d = nc.dram_tensor("onesm", [128, 128], bf16, kind="ExternalInput")
    b1_d = nc.dram_tensor("b1c", [128, 1], f32, kind="ExternalInput")
    posu_d = nc.dram_tensor("posu", [128, UC, 3], f32, kind="ExternalInput")
    offu_d = nc.dram_tensor("offu", [128, UC, 3], f32, kind="ExternalInput")
    stab_d = nc.dram_tensor("stab", [128, UC], f32, kind="ExternalOutput")
    fpos_d = nc.dram_tensor("fpos", [128, UC, 3], f32, kind="ExternalOutput")
    d2o_d = nc.dram_tensor("d2o_scratch", [128, UC], f32)
    d2n_d = nc.dram_tensor("d2n_scratch", [128, UC], f32)

    with tile.TileContext(nc) as tc:
        with (
            tc.tile_pool(name="singles", bufs=1) as singles,
            tc.tile_pool(name="persist", bufs=1) as persist,
            tc.tile_pool(name="stream", bufs=3) as stream,
            tc.tile_pool(name="work", bufs=3) as work,
            tc.tile_pool(name="psum_mm", bufs=8, space="PSUM") as psum_mm,
        ):
            w1t = singles.tile([128, 128], bf16, tag="w1t")
            w1b = singles.tile([128, 128], bf16, tag="w1b")
            w2 = singles.tile([128, 128], bf16, tag="w2")
            negI = singles.tile([128, 128], bf16, tag="negI")
            onesm = singles.tile([128, 128], bf16, tag="onesm")
            b1 = singles.tile([128, 1], f32, tag="b1")
            posu = singles.tile([128, UC, 3], f32, tag="posu")
            offu = singles.tile([128, UC, 3], f32, tag="offu")
            nc.sync.dma_start(out=w1t[:], in_=w1t_d[:])
            nc.sync.dma_start(out=w1b[:], in_=w1b_d[:])
            nc.sync.dma_start(out=w2[:], in_=w2_d[:])
            nc.sync.dma_start(out=negI[:], in_=negI_d[:])
            nc.sync.dma_start(out=onesm[:], in_=onesm_d[:])
            nc.sync.dma_start(out=b1[:], in_=b1_d[:])
            nc.sync.dma_start(out=posu[:], in_=posu_d[:])
            nc.sync.dma_start(out=offu[:], in_=offu_d[:])

            for t in range(NT):
                us = t * T
                sigb_t = stream.tile([128, T], bf16, tag="sigb")
                nc.sync.dma_start(out=sigb_t[:], in_=sigbT[:, us : us + T])
                loco_t = stream.tile([128, T], bf16, tag="loco")
                nc.sync.dma_start(out=loco_t[:], in_=locoldT[:, us : us + T])
                locn_t = stream.tile([128, T], bf16, tag="locn")
                nc.sync.dma_start(out=locn_t[:], in_=locnewT[:, us : us + T])

                # Layer 1, stationaries shared across both evals.
                preO = psum_mm.tile([128, T], f32, tag="mm")
                preN = psum_mm.tile([128, T], f32, tag="mm")
                nc.tensor.matmul(preO[:], lhsT=w1t[:], rhs=sigb_t[:], start=True, stop=False)
                nc.tensor.matmul(preN[:], lhsT=w1t[:], rhs=sigb_t[:], start=True, stop=False)
                nc.tensor.matmul(preO[:], lhsT=w1b[:], rhs=loco_t[:], start=False, stop=True)
                nc.tensor.matmul(preN[:], lhsT=w1b[:], rhs=locn_t[:], start=False, stop=True)

                hO = work.tile([128, T], bf16, tag="h")
                hN = work.tile([128, T], bf16, tag="h")
                nc.scalar.activation(hO[:], preO[:], AF.Tanh, bias=b1[:])
                nc.scalar.activation(hN[:], preN[:], AF.Tanh, bias=b1[:])

                # Layer 2 minus signatures: dps = W2.T @ h - sig*
                dpsO = psum_mm.tile([128, T], f32, tag="mm")
                dpsN = psum_mm.tile([128, T], f32, tag="mm")
                nc.tensor.matmul(dpsO[:], lhsT=w2[:], rhs=hO[:], start=True, stop=False)
                nc.tensor.matmul(dpsN[:], lhsT=w2[:], rhs=hN[:], start=True, stop=False)
                nc.tensor.matmul(dpsO[:], lhsT=negI[:], rhs=sigb_t[:], start=False, stop=True)
                nc.tensor.matmul(dpsN[:], lhsT=negI[:], rhs=sigb_t[:], start=False, stop=True)

                # Square on ScalarE (single PSUM read), bf16 out for the
                # all-ones reduction matmul.
                sqO = work.tile([128, T], bf16, tag="sq")
                sqN = work.tile([128, T], bf16, tag="sq")
                nc.scalar.activation(sqO[:], dpsO[:], AF.Square)
                nc.scalar.activation(sqN[:], dpsN[:], AF.Square)

                # d2 broadcast to all partitions via all-ones stationary.
                d2pO = psum_mm.tile([128, T], f32, tag="mm")
                d2pN = psum_mm.tile([128, T], f32, tag="mm")
                nc.tensor.matmul(d2pO[:], lhsT=onesm[:], rhs=sqO[:], start=True, stop=True)
                nc.tensor.matmul(d2pN[:], lhsT=onesm[:], rhs=sqN[:], start=True, stop=True)
                d2bO = work.tile([1, T], f32, tag="d2b")
                d2bN = work.tile([1, T], f32, tag="d2b")
                nc.vector.tensor_copy(d2bO[:], d2pO[0:1, :])
                nc.vector.tensor_copy(d2bN[:], d2pN[0:1, :])
                nc.sync.dma_start(out=d2o_d[2 * t : 2 * t + 2, :], in_=d2bO[0:1, :])
                nc.sync.dma_start(out=d2n_d[2 * t : 2 * t + 2, :], in_=d2bN[0:1, :])

            # Epilogue: accept, stability, final positions.
            d2o_sb = persist.tile([128, UC], f32, tag="d2o")
            d2n_sb = persist.tile([128, UC], f32, tag="d2n")
            acc_sb = persist.tile([128, UC], f32, tag="acc")
            min_sb = persist.tile([128, UC], f32, tag="min")
            stab_sb = persist.tile([128, UC], f32, tag="stab")
            fp_sb = persist.tile([128, UC, 3], f32, tag="fp")
            nc.sync.dma_start(out=d2o_sb[:], in_=d2o_d[:])
            nc.sync.dma_start(out=d2n_sb[:], in_=d2n_d[:])
            nc.vector.tensor_tensor(
                out=acc_sb[:], in0=d2n_sb[:], in1=d2o_sb[:], op=ALU.is_le
            )
            nc.vector.tensor_tensor(
                out=min_sb[:], in0=d2n_sb[:], in1=d2o_sb[:], op=ALU.min
            )
            nc.scalar.activation(stab_sb[:], min_sb[:], AF.Sqrt)
            nc.sync.dma_start(out=stab_d[:], in_=stab_sb[:])
            for c in range(3):
                nc.vector.tensor_tensor(
                    out=fp_sb[:, :, c], in0=offu[:, :, c], in1=acc_sb[:], op=ALU.mult
                )
                nc.vector.tensor_tensor(
                    out=fp_sb[:, :, c], in0=fp_sb[:, :, c], in1=posu[:, :, c], op=ALU.add
                )
            nc.sync.dma_start(out=fpos_d[:], in_=fp_sb[:])

    nc.finalize()
    return nc


def get_graph():
    global _GRAPH
    if _GRAPH is None:
        _GRAPH = _build_graph()
    return _GRAPH


def _locals_of(field, pos):
    pc = np.clip(pos.astype(np.int32), 0, G - 1)
    xx = np.clip(pc[:, 0:1] + _OI[None, :], 0, G - 1)
    yy = np.clip(pc[:, 1:2] + _OJ[None, :], 0, G - 1)
    zz = np.clip(pc[:, 2:3] + _OK[None, :], 0, G - 1)
    return field[xx, yy, zz]


def prepare_in_maps(universe_field, positions, signatures, offsets, W1, b1, W2, b2):
    field = np.asarray(universe_field, np.float32)
    pos = np.asarray(positions, np.float32)
    sig = np.asarray(signatures, np.float32)
    off = np.asarray(offsets, np.float32)
    W1 = np.asarray(W1, np.float32)
    b1 = np.asarray(b1, np.float32)
    W2 = np.asarray(W2, np.float32)
    b2 = np.asarray(b2, np.float32)
    test_pos = pos + off

    lo = _locals_of(field, pos)
    ln = _locals_of(field, test_pos)
    sigstar = sig - b2[None, :]

    bf = ml_dtypes.bfloat16
    w1top = np.ascontiguousarray(W1[:D]).astype(bf)
    w1bot = np.ascontiguousarray(W1[D:]).astype(bf)
    w2c = np.ascontiguousarray(W2).astype(bf)
    negI = (-np.eye(128, dtype=np.float32)).astype(bf)
    onesm = np.ones((128, 128), np.float32).astype(bf)
    b1c = b1.reshape(128, 1).astype(np.float32)

    in_maps = []
    for c in range(NCORES):
        sl = slice(c * NS, (c + 1) * NS)
        in_maps.append(
            {
                "sigbT": np.ascontiguousarray(sigstar[sl].T).astype(bf),
                "locoldT": np.ascontiguousarray(lo[sl].T).astype(bf),
                "locnewT": np.ascontiguousarray(ln[sl].T).astype(bf),
                "W1top": w1top,
                "W1bot": w1bot,
                "W2c": w2c,
                "negI": negI,
                "onesm": onesm,
                "b1c": b1c,
                "posu": np.ascontiguousarray(pos[sl].reshape(128, UC, 3)),
                "offu": np.ascontiguousarray(off[sl].reshape(128, UC, 3)),
            }
        )
    return in_maps


def collect_outputs(results):
    stab = np.empty(N, np.float32)
    fp = np.empty((N, 3), np.float32)
    for c in range(NCORES):
        r = results[c]
        stab[c * NS : (c + 1) * NS] = np.asarray(r["stab"]).reshape(NS)
        fp[c * NS : (c + 1) * NS] = np.asarray(r["fpos"]).reshape(NS, 3)
    return stab, fp


def kernel(universe_field, positions, signatures, offsets, W1, b1, W2, b2):
    from concourse.bass_utils import run_bass_kernel_spmd

    in_maps = prepare_in_maps(
        universe_field, positions, signatures, offsets, W1, b1, W2, b2
    )
    nc = get_graph()
    res = run_bass_kernel_spmd(nc, in_maps, core_ids=list(range(NCORES)))
    return collect_outputs(res.results)
